# revision 10
# baseline (speedup 1.0000x reference)
"""Trainium2 Bass kernel for nn_EnergyModel (irrepwise-MSE energy reduction).

Math (matches the reference):
    diff[t,q,d]  = descriptor[t,q,d] - query_feature[t,q,d]
    energy[t]    = sum_q a[q] * sum_d 2*w[group(d)] * diff[t,q,d]^2
    w[g]         = softplus(irrep_weight_logit[g]) / (ln2 * 192)
    energy[t]    = 100000.0 where any coord of T[t,4:7] lies outside ranges

Sharding: Nt=1024 poses split across 8 NeuronCores (128 poses per core); the
128 local poses sit on the SBUF partition axis so every DRAM read per
partition is one long contiguous burst.  query_attention / weights are
replicated.  Per core the two [128, 128*576] f32 operands are streamed in
q-chunks: one DVE subtract per chunk, then one ScalarE Square-activation with
per-partition accumulate per query column (the uniform irrep weight rides in
the activation scale), and a final fused multiply-reduce against attention on
DVE.  The O(Nt) range mask is applied on host after the gather.
"""

import math
import sys

import numpy as np

for _p in ("/opt/trn_rl_repo",):
    if _p not in sys.path:
        sys.path.insert(0, _p)

import concourse.bacc as bacc
import concourse.bass as bass
import concourse.mybir as mybir
from concourse.bass_utils import run_bass_kernel_spmd
from concourse.tile import TileContext

N_CORES = 8
NT, NQ, D = 1024, 128, 576
G = 192
LN2 = 0.6931471805599453
NT_LOC = NT // N_CORES  # 128 poses per core == SBUF partition count

# d-multiplicity per irrep group: 64 groups of l=0 (d=1), 64 of l=1 (d=3),
# 64 of l=2 (d=5) -> feature dim 576
_GROUP_DIMS = np.array([1] * 64 + [3] * 64 + [5] * 64)

_cache: dict = {}
_last_in_maps: list | None = None


def _build(act_scale: float, general: bool, qc: int) -> bass.Bass:
    """Build the per-core SPMD Bass program.

    act_scale: immediate scale for the Square activation (sqrt(w_bar) on the
        fast path, 1.0 on the general path where sqrt(w_d) is a tensor).
    general: multiply diff by a sqrt(w_d) broadcast tile (non-uniform logits).
    qc: queries per streamed chunk.
    """
    nchunks = NQ // qc
    F = qc * D
    f32 = mybir.dt.float32

    nc = bacc.Bacc(
        "TRN2", target_bir_lowering=False, debug=False, num_devices=N_CORES
    )
    desc = nc.declare_dram_parameter("desc", [NT_LOC, NQ * D], f32, isOutput=False)
    qf = nc.declare_dram_parameter("qf", [NT_LOC, NQ * D], f32, isOutput=False)
    attnb = nc.declare_dram_parameter("attnb", [NT_LOC, NQ], f32, isOutput=False)
    if general:
        wsq = nc.declare_dram_parameter("wsq", [NT_LOC, F], f32, isOutput=False)
    energy = nc.declare_dram_parameter("energy", [NT_LOC, 1], f32, isOutput=True)

    with TileContext(nc) as tc:
        with (
            tc.tile_pool(name="io", bufs=2) as io,
            tc.tile_pool(name="acc", bufs=1) as acc,
        ):
            s = acc.tile([NT_LOC, NQ], f32)
            attn_t = acc.tile([NT_LOC, NQ], f32)
            nc.sync.dma_start(out=attn_t[:], in_=attnb[:])
            if general:
                wsq_t = acc.tile([NT_LOC, F], f32)
                nc.sync.dma_start(out=wsq_t[:], in_=wsq[:])
            scratch = acc.tile([NT_LOC, D], f32)

            for c in range(nchunks):
                desc_t = io.tile([NT_LOC, F], f32, tag="desc")
                qf_t = io.tile([NT_LOC, F], f32, tag="qf")
                nc.sync.dma_start(out=desc_t[:], in_=desc[:, c * F : (c + 1) * F])
                nc.sync.dma_start(out=qf_t[:], in_=qf[:, c * F : (c + 1) * F])
                # diff (in place over the descriptor tile)
                nc.vector.tensor_tensor(
                    desc_t[:], desc_t[:], qf_t[:], mybir.AluOpType.subtract
                )
                if general:
                    nc.vector.tensor_tensor(
                        desc_t[:], desc_t[:], wsq_t[:], mybir.AluOpType.mult
                    )
                # s[t, q] = w_bar * sum_d diff^2   (per query column)
                for j in range(qc):
                    col = c * qc + j
                    nc.scalar.activation(
                        scratch[:],
                        desc_t[:, j * D : (j + 1) * D],
                        mybir.ActivationFunctionType.Square,
                        bias=0.0,
                        scale=float(act_scale),
                        accum_out=s[:, col : col + 1],
                    )

            # energy[t] = sum_q s[t,q] * (2*a[q])
            sa = acc.tile([NT_LOC, NQ], f32)
            e_t = acc.tile([NT_LOC, 1], f32)
            nc.vector.tensor_tensor(sa[:], s[:], attn_t[:], mybir.AluOpType.mult)
            nc.vector.tensor_reduce(
                e_t[:], sa[:], axis=mybir.AxisListType.X, op=mybir.AluOpType.add
            )
            nc.sync.dma_start(out=energy[:], in_=e_t[:])
    nc.finalize()  # Bacc.compile(): wait legalization, reg alloc, nop fusion
    return nc


def _softplus64(x: np.ndarray) -> np.ndarray:
    x = np.asarray(x, dtype=np.float64)
    return np.log1p(np.exp(-np.abs(x))) + np.maximum(x, 0.0)


def kernel(T, descriptor, query_feature, query_attention, irrep_weight_logit, ranges):
    descriptor = np.ascontiguousarray(np.asarray(descriptor), dtype=np.float32)
    query_feature = np.ascontiguousarray(np.asarray(query_feature), dtype=np.float32)
    a = np.asarray(query_attention, dtype=np.float64)
    w_group = _softplus64(irrep_weight_logit) / (LN2 * G)  # [192]

    uniform = bool(np.all(w_group == w_group[0]))
    if uniform:
        act_scale = math.sqrt(float(w_group[0]))
        qc, general = 16, False
        wsq_pat = None
    else:
        act_scale = 1.0
        qc, general = 8, True
        w_feat = np.repeat(w_group, _GROUP_DIMS)  # [576]
        wsq_pat = np.tile(np.sqrt(w_feat).astype(np.float32), qc)

    key = (general, qc, act_scale, None if wsq_pat is None else wsq_pat.tobytes())
    nc = _cache.get(key)
    if nc is None:
        nc = _build(act_scale, general, qc)
        _cache[key] = nc

    attnb = np.ascontiguousarray(
        np.broadcast_to((2.0 * a).astype(np.float32), (NT_LOC, NQ))
    )
    in_maps = []
    for i in range(N_CORES):
        m = {
            "desc": descriptor[i * NT_LOC : (i + 1) * NT_LOC].reshape(NT_LOC, NQ * D),
            "qf": query_feature[i * NT_LOC : (i + 1) * NT_LOC].reshape(NT_LOC, NQ * D),
            "attnb": attnb,
        }
        if general:
            m["wsq"] = np.ascontiguousarray(
                np.broadcast_to(wsq_pat, (NT_LOC, qc * D))
            )
        in_maps.append(m)

    global _last_in_maps
    _last_in_maps = in_maps
    res = run_bass_kernel_spmd(nc, in_maps, core_ids=list(range(N_CORES)))
    energy = np.concatenate([r["energy"][:, 0] for r in res.results])

    # host-side O(Nt) range mask
    X = np.asarray(T, dtype=np.float32)[:, 4:7]
    rg = np.asarray(ranges, dtype=np.float32)
    in_range = (rg[None, :, 1] >= X) & (X >= rg[None, :, 0])
    energy = np.where(
        np.any(~in_range, axis=-1), np.float32(100000.0), energy.astype(np.float32)
    )
    return energy.astype(np.float32)
